# revision 39
# baseline (speedup 1.0000x reference)
"""Trainium2 Bass kernel for nn_DownMP (GNN down-sampling message passing).

Contract: kernel(**inputs) takes the FULL unsharded numpy inputs and returns
the full outputs (field_lr, ei, ea) exactly as reference.reference() does.

Distribution strategy (per the METIS-style sharding hint):
  - LR node (cluster) space is range-partitioned across the 8 cores
    (core k owns clusters [k*8192, (k+1)*8192)).
  - HR nodes are partitioned to the core owning their cluster, so the
    segment-mean reduction is fully device-local (no collectives).
  - Edges are partitioned by (remapped) source LR node; the per-core edge
    lists are sorted/coalesced structurally on the host (int index metadata
    only); all floating point math (MLP, layernorm, segment means, edge
    attribute means) runs on device.

Device work per core (one SPMD NEFF on 8 NeuronCores):
  - 3-layer MLP (bf16 matmuls, f32 PSUM) with exact SELU via
    relu/exp composition, LayerNorm, one-hot segment-sum matmul,
    per-cluster mean, ln_g/ln_b applied post-reduction.
  - edge-attribute stream with duplicate-segment means.
"""

import json
import math
import os
import tempfile

import numpy as np
import ml_dtypes

import concourse.bass as bass
import concourse.bacc as bacc
import concourse.tile as tile
from concourse import mybir
from concourse.bass_utils import run_bass_kernel_spmd


def _pin_act_table_set():
    """Point walrus at a filtered act_info.json in which the only table set
    providing `exp` is natural_log_exp_and_others (which also has `ln`).
    Otherwise alternating Exp/Ln activations reload the ACT table RAMs on
    every switch (~1.3us each, hundreds of times)."""
    if os.environ.get("BASS_ACT_ROOT_JSON_PATH"):
        return
    try:
        from neuronxcc.driver.Job import Job
        from neuronxcc.driver.jobs.support.FindActInfo import findActInfoFile
        src = findActInfoFile(Job.getPackageDir(), "trainium2")
    except Exception:
        import neuronxcc
        src = os.path.join(os.path.dirname(neuronxcc.__file__),
                           "pwp", "pwp_bin_trainium", "act_info.json")
    try:
        info = json.load(open(src))
        for ent in info["act_func_sets"]:
            if ent["name"] != "natural_log_exp_and_others":
                ent["act"].pop("exp", None)
        dst = os.path.join(tempfile.mkdtemp(prefix="actinfo_"), "act_info.json")
        with open(dst, "w") as f:
            json.dump(info, f)
        os.environ["BASS_ACT_ROOT_JSON_PATH"] = dst
    except Exception:
        pass


# _pin_act_table_set()  # breaks NEFF load on this runtime

# ---------------- problem constants (hardcoded per spec) ----------------
N_HR = 262144
N_LR = 65536
E = 2097152
NMASK = 57344
F = 128
ED = 128
DE = 3
H = 128
DIN = ED + F  # 256

NCORE = 8
BPC = N_LR // NCORE          # buckets (LR clusters) per core = 8192
GROUPS = 64                  # bucket groups of 128 buckets per core
TPG = 5                      # 128-row tiles per bucket group (capacity 640 rows)
NP_ROWS = GROUPS * TPG * 128  # padded HR rows per core = 40960
NTILES = GROUPS * TPG        # 320

SEGC = 2080 * 128            # padded segments per core = 266240
DUPC = 256                   # max dup segments per core (layout 128x2)
MAXD = 8                     # max edges per dup segment handled

LAMBDA = 1.0507009873554805
ALPHA = 1.6732632423543772
LN_EPS = 1e-5

BF16 = mybir.dt.bfloat16
F32 = mybir.dt.float32

_COMPILED = {}


# ======================= device graph =======================

def build_graph():
    nc = bacc.Bacc(target_bir_lowering=False, debug=False)

    # ---- dram parameters ----
    xT = nc.dram_tensor("xT", [DIN, NP_ROWS], BF16, kind="ExternalInput")
    o_cm = nc.dram_tensor("o_cm", [128, NTILES], F32, kind="ExternalInput")
    cnt_cm = nc.dram_tensor("cnt_cm", [128, GROUPS], F32, kind="ExternalInput")
    w1 = nc.dram_tensor("w1", [DIN, H], BF16, kind="ExternalInput")
    w2 = nc.dram_tensor("w2", [H, H], BF16, kind="ExternalInput")   # pre-scaled by lambda on device? no: raw bf16
    w3 = nc.dram_tensor("w3", [H, H], BF16, kind="ExternalInput")
    # vecs columns: 0=b1, 1=b2, 2=b3, 3=ln_g, 4=ln_b  (each length H on partitions)
    vecs = nc.dram_tensor("vecs", [128, 8], F32, kind="ExternalInput")
    # rows: 0=iota(0..127) broadcast source, 1=ln_g, 2=ln_b  (values along free dim)
    vrow = nc.dram_tensor("vrow", [4, 128], F32, kind="ExternalInput")
    ident = nc.dram_tensor("ident", [128, 128], BF16, kind="ExternalInput")
    o_oh = nc.dram_tensor("o_oh", [128, NTILES, 128], BF16, kind="ExternalInput")
    ea_in = nc.dram_tensor("ea_in", [SEGC, DE], F32, kind="ExternalInput")
    dup_vals = nc.dram_tensor("dup_vals", [128, DUPC // 128, MAXD, DE], F32,
                              kind="ExternalInput")
    dup_cnt = nc.dram_tensor("dup_cnt", [128, DUPC // 128], F32, kind="ExternalInput")

    table = nc.dram_tensor("table", [BPC, H], F32, kind="ExternalOutput")
    ea_out = nc.dram_tensor("ea_out", [SEGC, DE], F32, kind="ExternalOutput")
    dup_out = nc.dram_tensor("dup_out", [128, DUPC // 128, DE], F32,
                             kind="ExternalOutput")

    DD = DUPC // 128  # 2

    with tile.TileContext(nc) as tc:
        with (
            tc.tile_pool(name="statics", bufs=1) as st,
            tc.tile_pool(name="xin", bufs=5) as xin,
            tc.tile_pool(name="mlp_ps", bufs=4, space="PSUM") as mlp_ps,
            tc.tile_pool(name="tr_ps", bufs=1, space="PSUM") as tr_ps,
            tc.tile_pool(name="seg_ps", bufs=2, space="PSUM") as seg_ps,
            tc.tile_pool(name="work", bufs=5) as wk,
            tc.tile_pool(name="stash", bufs=20) as stash,
            tc.tile_pool(name="tiny", bufs=6) as tiny,
            tc.tile_pool(name="out", bufs=4) as outp,
            tc.tile_pool(name="ea", bufs=2) as eap,
        ):
            # ---------- statics ----------
            w1t = st.tile([128, 2, H], BF16)   # two contraction chunks of W1
            nc.sync.dma_start(out=w1t[:, 0, :], in_=w1[0:128, :])
            nc.sync.dma_start(out=w1t[:, 1, :], in_=w1[128:256, :])
            w2t = st.tile([H, H], BF16)
            nc.sync.dma_start(out=w2t[:], in_=w2[:])
            w3t = st.tile([H, H], BF16)
            nc.sync.dma_start(out=w3t[:], in_=w3[:])
            vt = st.tile([128, 8], F32)
            nc.sync.dma_start(out=vt[:], in_=vecs[:])
            it = st.tile([128, 128], BF16)
            nc.sync.dma_start(out=it[:], in_=ident[:])
            oc = st.tile([128, NTILES], F32)
            nc.sync.dma_start(out=oc[:], in_=o_cm[:])
            cc = st.tile([128, GROUPS], F32)
            nc.sync.dma_start(out=cc[:], in_=cnt_cm[:])
            # iota row (0..127 along free), ln_g, ln_b broadcast to all partitions
            def bcast_row(i):
                row = vrow[i:i + 1, :]
                return bass.AP(tensor=row.tensor, offset=row.offset,
                               ap=[[0, 128]] + list(row.ap[1:]))

            iota_f = st.tile([128, 128], F32)
            nc.sync.dma_start(out=iota_f[:], in_=bcast_row(0))
            gT = st.tile([128, 128], F32)
            nc.sync.dma_start(out=gT[:], in_=bcast_row(1))
            bT = st.tile([128, 128], F32)
            nc.sync.dma_start(out=bT[:], in_=bcast_row(2))

            ones_col = st.tile([128, 1], BF16)
            nc.vector.memset(ones_col[:], 1.0)
            eps_col = st.tile([128, 1], F32)
            nc.vector.memset(eps_col[:], LN_EPS)

            # derived scalars/vectors (device-computed):
            # bias1e = b1 + ln(alpha); c2 = b2 - lambda*alpha*colsum(W2)
            # c2e = c2 + ln(alpha); c3 = b3 - lambda*alpha*colsum(W3)
            bias1 = st.tile([128, 1], F32)          # b1
            nc.vector.tensor_copy(out=bias1[:], in_=vt[:, 0:1])
            bias1e = st.tile([128, 1], F32)
            nc.vector.tensor_scalar_add(bias1e[:], vt[:, 0:1], math.log(ALPHA))

            cs_ps = seg_ps.tile([128, 1], F32, tag="seg")
            nc.tensor.matmul(out=cs_ps[:], lhsT=w2t[:], rhs=ones_col[:],
                             start=True, stop=True)
            c2 = st.tile([128, 1], F32)
            # c2 = b2 - lambda*alpha*colsum(W2)   (cs_ps holds colsum(W2))
            nc.vector.tensor_scalar(
                out=c2[:], in0=cs_ps[:], scalar1=-ALPHA, scalar2=None,
                op0=mybir.AluOpType.mult)
            nc.vector.tensor_tensor(out=c2[:], in0=c2[:], in1=vt[:, 1:2],
                                    op=mybir.AluOpType.add)
            c2e = st.tile([128, 1], F32)
            nc.vector.tensor_scalar_add(c2e[:], c2[:], math.log(ALPHA))
            c2a = st.tile([128, 1], F32)
            nc.vector.tensor_scalar_add(c2a[:], c2[:], ALPHA)

            cs_ps2 = seg_ps.tile([128, 1], F32, tag="seg")
            nc.tensor.matmul(out=cs_ps2[:], lhsT=w3t[:], rhs=ones_col[:],
                             start=True, stop=True)
            c3 = st.tile([128, 1], F32)
            nc.vector.tensor_scalar(
                out=c3[:], in0=cs_ps2[:], scalar1=-ALPHA, scalar2=None,
                op0=mybir.AluOpType.mult)
            nc.vector.tensor_tensor(out=c3[:], in0=c3[:], in1=vt[:, 2:3],
                                    op=mybir.AluOpType.add)

            # recip = 1 / max(cnt, 1)
            recip = st.tile([128, GROUPS], F32)
            nc.vector.tensor_scalar_max(recip[:], cc[:], 1.0)
            nc.vector.reciprocal(out=recip[:], in_=recip[:])

            # ---------- edge attr pass-through + dup means ----------
            EA_CH = 4
            ea_flat_in = ea_in[:].rearrange("(c r) d -> c (r d)", c=EA_CH)
            ea_flat_out = ea_out[:].rearrange("(c r) d -> c (r d)", c=EA_CH)
            rows_ch = SEGC // EA_CH  # rows per chunk
            for c in range(EA_CH):
                eat = eap.tile([128, rows_ch * DE // 128], F32, tag="ea")
                src_ap = ea_flat_in[c].rearrange("(p f) -> p f", p=128)
                dst_ap = ea_flat_out[c].rearrange("(p f) -> p f", p=128)
                nc.sync.dma_start(out=eat[:], in_=src_ap)
                nc.sync.dma_start(out=dst_ap, in_=eat[:])

            dv = tiny.tile([128, DD, MAXD, DE], F32, tag="dv")
            nc.sync.dma_start(out=dv[:], in_=dup_vals[:])
            dc = tiny.tile([128, DD], F32, tag="dc")
            nc.sync.dma_start(out=dc[:], in_=dup_cnt[:])
            dsum = tiny.tile([128, DD, DE], F32, tag="dsum")
            nc.vector.tensor_tensor(out=dsum[:], in0=dv[:, :, 0, :],
                                    in1=dv[:, :, 1, :], op=mybir.AluOpType.add)
            for j in range(2, MAXD):
                nc.vector.tensor_tensor(out=dsum[:], in0=dsum[:],
                                        in1=dv[:, :, j, :],
                                        op=mybir.AluOpType.add)
            drec = tiny.tile([128, DD], F32, tag="drec")
            nc.vector.tensor_scalar_max(drec[:], dc[:], 1.0)
            nc.vector.reciprocal(out=drec[:], in_=drec[:])
            dm = tiny.tile([128, DD, DE], F32, tag="dm")
            for j in range(DD):
                nc.vector.tensor_scalar(
                    out=dm[:, j, :], in0=dsum[:, j, :], scalar1=drec[:, j:j + 1],
                    scalar2=None, op0=mybir.AluOpType.mult)
            nc.sync.dma_start(out=dup_out[:], in_=dm[:])

            # ---------- main node-path loop ----------
            SUBS = [(0, 384), (384, 256)]  # sub-group (row offset, rows)
            SG = 8                          # groups per LN batch
            for gs in range(0, GROUPS, SG):
                mvb = tiny.tile([128, SG * TPG, 2], F32, tag="mvg")
                rowstash = []
                for gp in range(gs, gs + SG, 2):
                    pair = (gp, gp + 1)
                    XA, XB, H3S = {}, {}, {}
                    for g in pair:
                        gcol = g * TPG * 128
                        xag = xin.tile([128, 640], BF16, tag="xa")
                        xbg = xin.tile([128, 640], BF16, tag="xb")
                        nc.sync.dma_start(out=xag[:],
                                          in_=xT[0:128, gcol:gcol + 640])
                        nc.sync.dma_start(out=xbg[:],
                                          in_=xT[128:256, gcol:gcol + 640])
                        XA[g], XB[g] = xag, xbg
                        H3S[g] = []
                    for (soff, srows) in SUBS:
                        sl = slice(soff, soff + srows)
                        h1d, e1d, r1d, s1d = {}, {}, {}, {}
                        for g in pair:
                            h1 = mlp_ps.tile([128, 384], F32, tag="mlp")
                            nc.tensor.matmul(out=h1[:, :srows], lhsT=w1t[:, 0, :],
                                             rhs=XA[g][:, sl], start=True, stop=False)
                            nc.tensor.matmul(out=h1[:, :srows], lhsT=w1t[:, 1, :],
                                             rhs=XB[g][:, sl], start=False, stop=True)
                            h1d[g] = h1
                        for g in pair:
                            e1 = wk.tile([128, 384], BF16, tag="e1")
                            nc.scalar.activation(out=e1[:, :srows], in_=h1d[g][:, :srows],
                                                 func=mybir.ActivationFunctionType.Exp,
                                                 bias=bias1e[:], scale=1.0)
                            e1d[g] = e1
                        for g in pair:
                            r1 = wk.tile([128, 384], BF16, tag="r1")
                            nc.scalar.activation(out=r1[:, :srows], in_=h1d[g][:, :srows],
                                                 func=mybir.ActivationFunctionType.Relu,
                                                 bias=bias1[:], scale=1.0)
                            r1d[g] = r1
                        for g in pair:
                            s1 = wk.tile([128, 384], BF16, tag="s1")
                            nc.vector.tensor_scalar_min(s1[:, :srows], e1d[g][:, :srows], ALPHA)
                            s1d[g] = s1
                        for g in pair:
                            nc.vector.tensor_tensor(out=s1d[g][:, :srows], in0=s1d[g][:, :srows],
                                                    in1=r1d[g][:, :srows],
                                                    op=mybir.AluOpType.add)
                        h2d, e2d, r2d, s2d = {}, {}, {}, {}
                        for g in pair:
                            h2 = mlp_ps.tile([128, 384], F32, tag="mlp")
                            nc.tensor.matmul(out=h2[:, :srows], lhsT=w2t[:],
                                             rhs=s1d[g][:, :srows], start=True, stop=True)
                            h2d[g] = h2
                        for g in pair:
                            e2 = wk.tile([128, 384], BF16, tag="e1")
                            nc.scalar.activation(out=e2[:, :srows], in_=h2d[g][:, :srows],
                                                 func=mybir.ActivationFunctionType.Exp,
                                                 bias=c2e[:], scale=1.0)
                            e2d[g] = e2
                        for g in pair:
                            r2 = wk.tile([128, 384], BF16, tag="r1")
                            nc.vector.tensor_scalar(
                                out=r2[:, :srows], in0=h2d[g][:, :srows],
                                scalar1=c2a[:], scalar2=ALPHA,
                                op0=mybir.AluOpType.add, op1=mybir.AluOpType.max)
                            r2d[g] = r2
                        for g in pair:
                            s2 = wk.tile([128, 384], BF16, tag="s1")
                            nc.vector.tensor_tensor(out=s2[:, :srows], in0=e2d[g][:, :srows],
                                                    in1=r2d[g][:, :srows],
                                                    op=mybir.AluOpType.min)
                            s2d[g] = s2
                        h3d = {}
                        for g in pair:
                            h3 = mlp_ps.tile([128, 384], F32, tag="mlp")
                            nc.tensor.matmul(out=h3[:, :srows], lhsT=w3t[:],
                                             rhs=s2d[g][:, :srows], start=True, stop=True)
                            h3d[g] = h3
                        for g in pair:
                            h3s = wk.tile([128, 384], BF16, tag="h3s")
                            nc.vector.tensor_scalar(
                                out=h3s[:, :srows], in0=h3d[g][:, :srows],
                                scalar1=c3[:], scalar2=None, op0=mybir.AluOpType.add)
                            H3S[g].append((h3s, srows))

                    # transpose to row-major, layernorm stats, stash bf16 rows
                    for g in pair:
                        tra = tr_ps.tile([128, 384], F32, tag="tra")
                        trb = tr_ps.tile([128, 256], F32, tag="trb")
                        trs = [tra[:, 0:128], tra[:, 128:256], tra[:, 256:384],
                               trb[:, 0:128], trb[:, 128:256]]
                        for t in range(TPG):
                            sub_i, sub_off = (0, t * 128) if t < 3 else (1, (t - 3) * 128)
                            h3s, _ = H3S[g][sub_i]
                            hsl = h3s[:, sub_off:sub_off + 128]
                            nc.tensor.matmul(out=trs[t], lhsT=hsl, rhs=it[:],
                                             start=True, stop=True)
                            stt = tiny.tile([128, 6], F32, tag="bn")
                            nc.vector.bn_stats(out=stt[:], in_=trs[t])
                            nc.vector.bn_aggr(out=mvb[:, (g - gs) * TPG + t, :],
                                              in_=stt[:])
                        ha = stash.tile([128, 384], BF16, tag="ha")
                        nc.scalar.copy(out=ha[:], in_=tra[:])
                        hb = stash.tile([128, 256], BF16, tag="hb")
                        nc.scalar.copy(out=hb[:], in_=trb[:])
                        rowstash.append((ha, hb))

                # batched rstd / mu*rstd for the supergroup
                lnv = tiny.tile([128, SG * TPG], F32, tag="lnv")
                nc.scalar.activation(out=lnv[:], in_=mvb[:, :, 1],
                                     func=mybir.ActivationFunctionType.Ln,
                                     bias=eps_col[:], scale=1.0)
                rstd = tiny.tile([128, SG * TPG], F32, tag="rstd")
                nc.scalar.activation(out=rstd[:], in_=lnv[:],
                                     func=mybir.ActivationFunctionType.Exp,
                                     bias=0.0, scale=-0.5)
                nmr = tiny.tile([128, SG * TPG], F32, tag="nmr")
                nc.vector.tensor_tensor(out=nmr[:], in0=mvb[:, :, 0],
                                        in1=rstd[:], op=mybir.AluOpType.mult)
                nmrn = tiny.tile([128, SG * TPG], F32, tag="nmrn")
                nc.vector.tensor_scalar_mul(nmrn[:], nmr[:], -1.0)

                # apply layernorm, one-hot segment matmul, per-bucket mean
                for gi0 in range(0, SG, 2):
                    gis = (gi0, gi0 + 1)
                    segd, ohd = {}, {}
                    for gi in gis:
                        g = gs + gi
                        seg_t = seg_ps.tile([128, H], F32, tag="seg")
                        segd[gi] = seg_t
                        ohg = xin.tile([128, TPG, 128], BF16, tag="ohg")
                        nc.sync.dma_start(out=ohg[:],
                                          in_=o_oh[:, g * TPG:(g + 1) * TPG, :])
                        ohd[gi] = ohg
                    for t in range(TPG):
                        yd = {}
                        for gi in gis:
                            col = gi * TPG + t
                            ha, hb = rowstash[gi]
                            src_ = (ha[:, t * 128:(t + 1) * 128] if t < 3
                                    else hb[:, (t - 3) * 128:(t - 2) * 128])
                            y = wk.tile([128, 128], BF16, tag="y")
                            if t % 2 == 0:
                                nc.scalar.activation(
                                    out=y[:], in_=src_,
                                    func=mybir.ActivationFunctionType.Identity,
                                    bias=nmrn[:, col:col + 1],
                                    scale=rstd[:, col:col + 1])
                            else:
                                nc.gpsimd.tensor_scalar(
                                    out=y[:], in0=src_,
                                    scalar1=rstd[:, col:col + 1],
                                    scalar2=nmr[:, col:col + 1],
                                    op0=mybir.AluOpType.mult,
                                    op1=mybir.AluOpType.subtract)
                            yd[gi] = y
                        for gi in gis:
                            nc.tensor.matmul(out=segd[gi][:], lhsT=ohd[gi][:, t, :],
                                             rhs=yd[gi][:],
                                             start=(t == 0), stop=(t == TPG - 1))
                    for gi in gis:
                        g = gs + gi
                        m1 = outp.tile([128, H], F32, tag="m1")
                        nc.vector.tensor_scalar(
                            out=m1[:], in0=segd[gi][:], scalar1=recip[:, g:g + 1],
                            scalar2=None, op0=mybir.AluOpType.mult)
                        # (kept on DVE: PSUM source; GPSIMD cannot read PSUM)
                        nc.gpsimd.tensor_tensor(out=m1[:], in0=m1[:], in1=gT[:],
                                                op=mybir.AluOpType.mult)
                        nc.gpsimd.tensor_tensor(out=m1[:], in0=m1[:], in1=bT[:],
                                                op=mybir.AluOpType.add)
                        nc.sync.dma_start(out=table[g * 128:(g + 1) * 128, :],
                                          in_=m1[:])

    nc.finalize()
    return nc


# ======================= host prep =======================

def _prep(e, field, edge_attr, cluster, mask_idx, idx_hr_to_lr, edge_index,
          W1, b1, W2, b2, W3, b3, ln_g, ln_b):
    bf16 = ml_dtypes.bfloat16
    cluster = np.asarray(cluster).astype(np.int64)

    # ----- node partition: sort HR rows by cluster, group into bucket groups
    order = np.argsort(cluster, kind="stable")
    csort = cluster[order]
    # boundaries of each 128-bucket group per core: group id = cluster >> 7
    gid = csort >> 7  # 0..511 global groups (64 per core)
    gstart = np.searchsorted(gid, np.arange(NCORE * GROUPS))
    gend = np.searchsorted(gid, np.arange(NCORE * GROUPS) + 1)
    glen = gend - gstart
    if glen.max() > TPG * 128:
        raise RuntimeError(f"bucket group overflow: {glen.max()} > {TPG*128}")

    perm = np.zeros((NCORE, NP_ROWS), dtype=np.int64)
    valid = np.zeros((NCORE, NP_ROWS), dtype=bool)
    o_all = np.full((NCORE, NP_ROWS), -1.0, dtype=np.float32)
    for k in range(NCORE):
        for g in range(GROUPS):
            gg = k * GROUPS + g
            n = glen[gg]
            base = g * TPG * 128
            rows = order[gstart[gg]:gend[gg]]
            perm[k, base:base + n] = rows
            valid[k, base:base + n] = True
            o_all[k, base:base + n] = (csort[gstart[gg]:gend[gg]]
                                       - (gg << 7)).astype(np.float32)

    cnt = np.bincount(cluster, minlength=N_LR).astype(np.float32)

    # per-core inputs
    e = np.asarray(e, dtype=np.float32)
    field = np.asarray(field, dtype=np.float32)
    ins = []
    w1b = np.asarray(W1, dtype=np.float32).astype(bf16)
    w2b = (LAMBDA * np.asarray(W2, dtype=np.float32)).astype(bf16)
    w3b = (LAMBDA * np.asarray(W3, dtype=np.float32)).astype(bf16)
    vecs = np.zeros((128, 8), dtype=np.float32)
    vecs[:, 0] = b1
    vecs[:, 1] = b2
    vecs[:, 2] = b3
    vecs[:, 3] = ln_g
    vecs[:, 4] = ln_b
    vrow = np.zeros((4, 128), dtype=np.float32)
    vrow[0] = np.arange(128, dtype=np.float32)
    vrow[1] = ln_g
    vrow[2] = ln_b
    ident = np.eye(128, dtype=np.float32).astype(bf16)

    # ----- edge partition: remap, drop self loops, global lexsort, coalesce
    ei0 = np.asarray(edge_index[0], dtype=np.int64)
    ei1 = np.asarray(edge_index[1], dtype=np.int64)
    idx_map = np.asarray(idx_hr_to_lr, dtype=np.int64)
    src = idx_map[ei0]
    dst = idx_map[ei1]
    keep = src != dst
    sv, dv_, eav = src[keep], dst[keep], np.asarray(edge_attr, np.float32)[keep]
    eorder = np.lexsort((dv_, sv))
    ss, ds = sv[eorder], dv_[eorder]
    ea_s = eav[eorder]
    newseg = np.empty(len(ss), dtype=bool)
    newseg[0] = True
    np.logical_or(ss[1:] != ss[:-1], ds[1:] != ds[:-1], out=newseg[1:])
    first = np.nonzero(newseg)[0]
    nseg = len(first)
    seglen = np.diff(np.append(first, len(ss)))
    seg_src = ss[first]
    seg_dst = ds[first]
    # split segments by owning core (seg_src sorted ascending)
    seg_cut = np.searchsorted(seg_src, np.arange(NCORE + 1) * BPC)

    ea_in = np.zeros((NCORE, SEGC, DE), dtype=np.float32)
    dup_vals = np.zeros((NCORE, 128, DUPC // 128, MAXD, DE), dtype=np.float32)
    dup_cnt = np.zeros((NCORE, 128, DUPC // 128), dtype=np.float32)
    dup_rows = []  # per core: local seg indices of dup segments (order = layout)
    nseg_k = np.zeros(NCORE, dtype=np.int64)
    if seglen.max() > MAXD:
        raise RuntimeError(f"dup segment too long: {seglen.max()} > {MAXD}")
    for k in range(NCORE):
        s0, s1 = seg_cut[k], seg_cut[k + 1]
        nk = s1 - s0
        nseg_k[k] = nk
        if nk > SEGC:
            raise RuntimeError(f"segment overflow core {k}: {nk} > {SEGC}")
        ea_in[k, :nk] = ea_s[first[s0:s1]]
        dl = np.nonzero(seglen[s0:s1] >= 2)[0]  # local seg ids of dups
        if len(dl) > DUPC:
            raise RuntimeError(f"dup overflow core {k}: {len(dl)} > {DUPC}")
        dup_rows.append(dl)
        for j, lseg in enumerate(dl):
            gseg = s0 + lseg
            st_e = first[gseg]
            ln = seglen[gseg]
            p, q = j % 128, j // 128
            dup_vals[k, p, q, :ln] = ea_s[st_e:st_e + ln]
            dup_cnt[k, p, q] = ln
    edge_meta = dict(nseg_k=nseg_k, seg_src=seg_src, seg_dst=seg_dst,
                     seg_cut=seg_cut, dup_rows=dup_rows)

    for k in range(NCORE):
        pk = perm[k]
        x = np.concatenate([e[pk], field[pk]], axis=1)  # (NP_ROWS, 256)
        x[~valid[k]] = 0.0
        xT = np.ascontiguousarray(x.T).astype(bf16)
        o_cm = np.ascontiguousarray(
            o_all[k].reshape(NTILES, 128).T).astype(np.float32)
        o_oh = np.ascontiguousarray(
            (o_cm[:, :, None] == np.arange(128, dtype=np.float32)[None, None, :])
        ).astype(ml_dtypes.bfloat16)
        cnt_k = cnt[k * BPC:(k + 1) * BPC]
        cnt_cm = np.ascontiguousarray(
            cnt_k.reshape(GROUPS, 128).T).astype(np.float32)
        ins.append({
            "xT": xT, "o_cm": o_cm, "cnt_cm": cnt_cm, "o_oh": o_oh,
            "w1": w1b, "w2": w2b, "w3": w3b,
            "vecs": vecs, "vrow": vrow, "ident": ident,
            "ea_in": ea_in[k], "dup_vals": dup_vals[k], "dup_cnt": dup_cnt[k],
        })
    return ins, edge_meta


# ======================= entry point =======================

def kernel(e, field, edge_attr, cluster, mask_idx, idx_hr_to_lr, edge_index,
           W1, b1, W2, b2, W3, b3, ln_g, ln_b):
    if "nc" not in _COMPILED:
        _COMPILED["nc"] = build_graph()
    nc = _COMPILED["nc"]

    ins, em = _prep(e, field, edge_attr, cluster, mask_idx, idx_hr_to_lr,
                    edge_index, W1, b1, W2, b2, W3, b3, ln_g, ln_b)

    rr = run_bass_kernel_spmd(nc, ins, list(range(NCORE)))
    _COMPILED["last_exec_time_ns"] = rr.exec_time_ns
    res = rr.results

    # ----- unshard: field_lr via mask gather over the per-core tables
    mask_idx = np.asarray(mask_idx, dtype=np.int64)
    field_lr = np.empty((NMASK, H), dtype=np.float32)
    mcut = np.searchsorted(mask_idx, np.arange(NCORE + 1) * BPC)
    for k in range(NCORE):
        mk = mask_idx[mcut[k]:mcut[k + 1]] - k * BPC
        field_lr[mcut[k]:mcut[k + 1]] = res[k]["table"][mk]

    # ----- unshard: ei / ea
    nseg_k = em["nseg_k"]
    nseg = int(nseg_k.sum())
    ei = np.full((2, E), -1, dtype=np.int32)
    ei[0, :nseg] = em["seg_src"]
    ei[1, :nseg] = em["seg_dst"]
    ea = np.zeros((E, DE), dtype=np.float32)
    off = 0
    for k in range(NCORE):
        nk = int(nseg_k[k])
        ea_k = res[k]["ea_out"][:nk].copy()
        dl = em["dup_rows"][k]
        if len(dl):
            dm = res[k]["dup_out"]  # (128, DUPC//128, DE)
            j = np.arange(len(dl))
            ea_k[dl] = dm[j % 128, j // 128]
        ea[off:off + nk] = ea_k
        off += nk

    return field_lr, ei, ea


# revision 40
# speedup vs baseline: 1.4654x; 1.4654x over previous
"""Trainium2 Bass kernel for nn_DownMP (GNN down-sampling message passing).

Contract: kernel(**inputs) takes the FULL unsharded numpy inputs and returns
the full outputs (field_lr, ei, ea) exactly as reference.reference() does.

Distribution strategy (per the METIS-style sharding hint):
  - LR node (cluster) space is range-partitioned across the 8 cores
    (core k owns clusters [k*8192, (k+1)*8192)).
  - HR nodes are partitioned to the core owning their cluster, so the
    segment-mean reduction is fully device-local (no collectives).
  - Edges are partitioned by (remapped) source LR node; the per-core edge
    lists are sorted/coalesced structurally on the host (int index metadata
    only); all floating point math (MLP, layernorm, segment means, edge
    attribute means) runs on device.

Device work per core (one SPMD NEFF on 8 NeuronCores):
  - 3-layer MLP (bf16 matmuls, f32 PSUM) with exact SELU via
    relu/exp composition, LayerNorm, one-hot segment-sum matmul,
    per-cluster mean, ln_g/ln_b applied post-reduction.
  - edge-attribute stream with duplicate-segment means.
"""

import json
import math
import os
import tempfile

import numpy as np
import ml_dtypes

import concourse.bass as bass
import concourse.bacc as bacc
import concourse.tile as tile
from concourse import mybir
from concourse.bass_utils import run_bass_kernel_spmd


def _pin_act_table_set():
    """Point walrus at a filtered act_info.json in which the only table set
    providing `exp` is natural_log_exp_and_others (which also has `ln`).
    Otherwise alternating Exp/Ln activations reload the ACT table RAMs on
    every switch (~1.3us each, hundreds of times)."""
    if os.environ.get("BASS_ACT_ROOT_JSON_PATH"):
        return
    try:
        from neuronxcc.driver.Job import Job
        from neuronxcc.driver.jobs.support.FindActInfo import findActInfoFile
        src = findActInfoFile(Job.getPackageDir(), "trainium2")
    except Exception:
        import neuronxcc
        src = os.path.join(os.path.dirname(neuronxcc.__file__),
                           "pwp", "pwp_bin_trainium", "act_info.json")
    try:
        info = json.load(open(src))
        for ent in info["act_func_sets"]:
            if ent["name"] != "natural_log_exp_and_others":
                ent["act"].pop("exp", None)
        dst = os.path.join(tempfile.mkdtemp(prefix="actinfo_"), "act_info.json")
        with open(dst, "w") as f:
            json.dump(info, f)
        os.environ["BASS_ACT_ROOT_JSON_PATH"] = dst
    except Exception:
        pass


# _pin_act_table_set()  # breaks NEFF load on this runtime

# ---------------- problem constants (hardcoded per spec) ----------------
N_HR = 262144
N_LR = 65536
E = 2097152
NMASK = 57344
F = 128
ED = 128
DE = 3
H = 128
DIN = ED + F  # 256

NCORE = 8
BPC = N_LR // NCORE          # buckets (LR clusters) per core = 8192
GROUPS = 64                  # bucket groups of 128 buckets per core
TPG = 5                      # 128-row tiles per bucket group (capacity 640 rows)
NP_ROWS = GROUPS * TPG * 128  # padded HR rows per core = 40960
NTILES = GROUPS * TPG        # 320

SEGC = 2080 * 128            # padded segments per core = 266240
DUPC = 256                   # max dup segments per core (layout 128x2)
MAXD = 8                     # max edges per dup segment handled

LAMBDA = 1.0507009873554805
ALPHA = 1.6732632423543772
LN_EPS = 1e-5

BF16 = mybir.dt.bfloat16
F32 = mybir.dt.float32

_COMPILED = {}


# ======================= device graph =======================

def build_graph():
    nc = bacc.Bacc(target_bir_lowering=False, debug=False)

    # ---- dram parameters ----
    xT = nc.dram_tensor("xT", [DIN, NP_ROWS], BF16, kind="ExternalInput")
    o_cm = nc.dram_tensor("o_cm", [128, NTILES], F32, kind="ExternalInput")
    cnt_cm = nc.dram_tensor("cnt_cm", [128, GROUPS], F32, kind="ExternalInput")
    w1 = nc.dram_tensor("w1", [DIN, H], BF16, kind="ExternalInput")
    w2 = nc.dram_tensor("w2", [H, H], BF16, kind="ExternalInput")   # pre-scaled by lambda on device? no: raw bf16
    w3 = nc.dram_tensor("w3", [H, H], BF16, kind="ExternalInput")
    # vecs columns: 0=b1, 1=b2, 2=b3, 3=ln_g, 4=ln_b  (each length H on partitions)
    vecs = nc.dram_tensor("vecs", [128, 8], F32, kind="ExternalInput")
    # rows: 0=iota(0..127) broadcast source, 1=ln_g, 2=ln_b  (values along free dim)
    vrow = nc.dram_tensor("vrow", [4, 128], F32, kind="ExternalInput")
    ident = nc.dram_tensor("ident", [128, 128], BF16, kind="ExternalInput")
    o_oh = nc.dram_tensor("o_oh", [128, NTILES, 128], BF16, kind="ExternalInput")
    ea_in = nc.dram_tensor("ea_in", [SEGC, DE], F32, kind="ExternalInput")
    dup_vals = nc.dram_tensor("dup_vals", [128, DUPC // 128, MAXD, DE], F32,
                              kind="ExternalInput")
    dup_cnt = nc.dram_tensor("dup_cnt", [128, DUPC // 128], F32, kind="ExternalInput")

    table = nc.dram_tensor("table", [BPC, H], F32, kind="ExternalOutput")
    ea_out = nc.dram_tensor("ea_out", [SEGC, DE], F32, kind="ExternalOutput")
    dup_out = nc.dram_tensor("dup_out", [128, DUPC // 128, DE], F32,
                             kind="ExternalOutput")

    DD = DUPC // 128  # 2

    with tile.TileContext(nc) as tc:
        with (
            tc.tile_pool(name="statics", bufs=1) as st,
            tc.tile_pool(name="xin", bufs=5) as xin,
            tc.tile_pool(name="mlp_ps", bufs=4, space="PSUM") as mlp_ps,
            tc.tile_pool(name="tr_ps", bufs=1, space="PSUM") as tr_ps,
            tc.tile_pool(name="seg_ps", bufs=2, space="PSUM") as seg_ps,
            tc.tile_pool(name="work", bufs=5) as wk,
            tc.tile_pool(name="stash", bufs=20) as stash,
            tc.tile_pool(name="tiny", bufs=6) as tiny,
            tc.tile_pool(name="out", bufs=4) as outp,
            tc.tile_pool(name="ea", bufs=2) as eap,
        ):
            # ---------- statics ----------
            w1t = st.tile([128, 2, H], BF16)   # two contraction chunks of W1
            nc.sync.dma_start(out=w1t[:, 0, :], in_=w1[0:128, :])
            nc.sync.dma_start(out=w1t[:, 1, :], in_=w1[128:256, :])
            w2t = st.tile([H, H], BF16)
            nc.sync.dma_start(out=w2t[:], in_=w2[:])
            w3t = st.tile([H, H], BF16)
            nc.sync.dma_start(out=w3t[:], in_=w3[:])
            vt = st.tile([128, 8], F32)
            nc.sync.dma_start(out=vt[:], in_=vecs[:])
            it = st.tile([128, 128], BF16)
            nc.sync.dma_start(out=it[:], in_=ident[:])
            oc = st.tile([128, NTILES], F32)
            nc.sync.dma_start(out=oc[:], in_=o_cm[:])
            cc = st.tile([128, GROUPS], F32)
            nc.sync.dma_start(out=cc[:], in_=cnt_cm[:])
            # iota row (0..127 along free), ln_g, ln_b broadcast to all partitions
            def bcast_row(i):
                row = vrow[i:i + 1, :]
                return bass.AP(tensor=row.tensor, offset=row.offset,
                               ap=[[0, 128]] + list(row.ap[1:]))

            iota_f = st.tile([128, 128], F32)
            nc.sync.dma_start(out=iota_f[:], in_=bcast_row(0))
            gT = st.tile([128, 128], F32)
            nc.sync.dma_start(out=gT[:], in_=bcast_row(1))
            bT = st.tile([128, 128], F32)
            nc.sync.dma_start(out=bT[:], in_=bcast_row(2))

            ones_col = st.tile([128, 1], BF16)
            nc.vector.memset(ones_col[:], 1.0)
            eps_col = st.tile([128, 1], F32)
            nc.vector.memset(eps_col[:], LN_EPS)

            # derived scalars/vectors (device-computed):
            # bias1e = b1 + ln(alpha); c2 = b2 - lambda*alpha*colsum(W2)
            # c2e = c2 + ln(alpha); c3 = b3 - lambda*alpha*colsum(W3)
            bias1 = st.tile([128, 1], F32)          # b1
            nc.vector.tensor_copy(out=bias1[:], in_=vt[:, 0:1])
            bias1e = st.tile([128, 1], F32)
            nc.vector.tensor_scalar_add(bias1e[:], vt[:, 0:1], math.log(ALPHA))

            cs_ps = seg_ps.tile([128, 1], F32, tag="seg")
            nc.tensor.matmul(out=cs_ps[:], lhsT=w2t[:], rhs=ones_col[:],
                             start=True, stop=True)
            c2 = st.tile([128, 1], F32)
            # c2 = b2 - lambda*alpha*colsum(W2)   (cs_ps holds colsum(W2))
            nc.vector.tensor_scalar(
                out=c2[:], in0=cs_ps[:], scalar1=-ALPHA, scalar2=None,
                op0=mybir.AluOpType.mult)
            nc.vector.tensor_tensor(out=c2[:], in0=c2[:], in1=vt[:, 1:2],
                                    op=mybir.AluOpType.add)
            c2e = st.tile([128, 1], F32)
            nc.vector.tensor_scalar_add(c2e[:], c2[:], math.log(ALPHA))
            c2a = st.tile([128, 1], F32)
            nc.vector.tensor_scalar_add(c2a[:], c2[:], ALPHA)

            cs_ps2 = seg_ps.tile([128, 1], F32, tag="seg")
            nc.tensor.matmul(out=cs_ps2[:], lhsT=w3t[:], rhs=ones_col[:],
                             start=True, stop=True)
            c3 = st.tile([128, 1], F32)
            nc.vector.tensor_scalar(
                out=c3[:], in0=cs_ps2[:], scalar1=-ALPHA, scalar2=None,
                op0=mybir.AluOpType.mult)
            nc.vector.tensor_tensor(out=c3[:], in0=c3[:], in1=vt[:, 2:3],
                                    op=mybir.AluOpType.add)

            # recip = 1 / max(cnt, 1)
            recip = st.tile([128, GROUPS], F32)
            nc.vector.tensor_scalar_max(recip[:], cc[:], 1.0)
            nc.vector.reciprocal(out=recip[:], in_=recip[:])

            # ---------- edge attr pass-through + dup means ----------
            EA_CH = 4
            ea_flat_in = ea_in[:].rearrange("(c r) d -> c (r d)", c=EA_CH)
            ea_flat_out = ea_out[:].rearrange("(c r) d -> c (r d)", c=EA_CH)
            rows_ch = SEGC // EA_CH  # rows per chunk
            for c in range(EA_CH):
                eat = eap.tile([128, rows_ch * DE // 128], F32, tag="ea")
                src_ap = ea_flat_in[c].rearrange("(p f) -> p f", p=128)
                dst_ap = ea_flat_out[c].rearrange("(p f) -> p f", p=128)
                nc.sync.dma_start(out=eat[:], in_=src_ap)
                nc.sync.dma_start(out=dst_ap, in_=eat[:])

            dv = tiny.tile([128, DD, MAXD, DE], F32, tag="dv")
            nc.sync.dma_start(out=dv[:], in_=dup_vals[:])
            dc = tiny.tile([128, DD], F32, tag="dc")
            nc.sync.dma_start(out=dc[:], in_=dup_cnt[:])
            dsum = tiny.tile([128, DD, DE], F32, tag="dsum")
            nc.vector.tensor_tensor(out=dsum[:], in0=dv[:, :, 0, :],
                                    in1=dv[:, :, 1, :], op=mybir.AluOpType.add)
            for j in range(2, MAXD):
                nc.vector.tensor_tensor(out=dsum[:], in0=dsum[:],
                                        in1=dv[:, :, j, :],
                                        op=mybir.AluOpType.add)
            drec = tiny.tile([128, DD], F32, tag="drec")
            nc.vector.tensor_scalar_max(drec[:], dc[:], 1.0)
            nc.vector.reciprocal(out=drec[:], in_=drec[:])
            dm = tiny.tile([128, DD, DE], F32, tag="dm")
            for j in range(DD):
                nc.vector.tensor_scalar(
                    out=dm[:, j, :], in0=dsum[:, j, :], scalar1=drec[:, j:j + 1],
                    scalar2=None, op0=mybir.AluOpType.mult)
            nc.sync.dma_start(out=dup_out[:], in_=dm[:])

            # ---------- main node-path loop ----------
            SUBS = [(0, 384), (384, 256)]  # sub-group (row offset, rows)
            SG = 8                          # groups per LN batch
            for gs in range(0, GROUPS, SG):
                mvb = tiny.tile([128, SG * TPG, 2], F32, tag="mvg")
                rowstash = []
                for gp in range(gs, gs + SG, 2):
                    pair = (gp, gp + 1)
                    XA, XB, H3S = {}, {}, {}
                    for g in pair:
                        gcol = g * TPG * 128
                        xag = xin.tile([128, 640], BF16, tag="xa")
                        xbg = xin.tile([128, 640], BF16, tag="xb")
                        nc.sync.dma_start(out=xag[:],
                                          in_=xT[0:128, gcol:gcol + 640])
                        nc.sync.dma_start(out=xbg[:],
                                          in_=xT[128:256, gcol:gcol + 640])
                        XA[g], XB[g] = xag, xbg
                        H3S[g] = []
                    for (soff, srows) in SUBS:
                        sl = slice(soff, soff + srows)
                        h1d, e1d, r1d, s1d = {}, {}, {}, {}
                        for g in pair:
                            h1 = mlp_ps.tile([128, 384], F32, tag="mlp")
                            nc.tensor.matmul(out=h1[:, :srows], lhsT=w1t[:, 0, :],
                                             rhs=XA[g][:, sl], start=True, stop=False)
                            nc.tensor.matmul(out=h1[:, :srows], lhsT=w1t[:, 1, :],
                                             rhs=XB[g][:, sl], start=False, stop=True)
                            h1d[g] = h1
                        for g in pair:
                            e1 = wk.tile([128, 384], BF16, tag="e1")
                            nc.scalar.activation(out=e1[:, :srows], in_=h1d[g][:, :srows],
                                                 func=mybir.ActivationFunctionType.Exp,
                                                 bias=bias1e[:], scale=1.0)
                            e1d[g] = e1
                        for g in pair:
                            r1 = wk.tile([128, 384], BF16, tag="r1")
                            nc.scalar.activation(out=r1[:, :srows], in_=h1d[g][:, :srows],
                                                 func=mybir.ActivationFunctionType.Relu,
                                                 bias=bias1[:], scale=1.0)
                            r1d[g] = r1
                        for g in pair:
                            s1 = wk.tile([128, 384], BF16, tag="s1")
                            nc.vector.tensor_scalar_min(s1[:, :srows], e1d[g][:, :srows], ALPHA)
                            s1d[g] = s1
                        for g in pair:
                            nc.vector.tensor_tensor(out=s1d[g][:, :srows], in0=s1d[g][:, :srows],
                                                    in1=r1d[g][:, :srows],
                                                    op=mybir.AluOpType.add)
                        h2d, e2d, r2d, s2d = {}, {}, {}, {}
                        for g in pair:
                            h2 = mlp_ps.tile([128, 384], F32, tag="mlp")
                            nc.tensor.matmul(out=h2[:, :srows], lhsT=w2t[:],
                                             rhs=s1d[g][:, :srows], start=True, stop=True)
                            h2d[g] = h2
                        for g in pair:
                            e2 = wk.tile([128, 384], BF16, tag="e1")
                            nc.scalar.activation(out=e2[:, :srows], in_=h2d[g][:, :srows],
                                                 func=mybir.ActivationFunctionType.Exp,
                                                 bias=c2e[:], scale=1.0)
                            e2d[g] = e2
                        for g in pair:
                            r2 = wk.tile([128, 384], BF16, tag="r1")
                            nc.vector.tensor_scalar(
                                out=r2[:, :srows], in0=h2d[g][:, :srows],
                                scalar1=c2a[:], scalar2=ALPHA,
                                op0=mybir.AluOpType.add, op1=mybir.AluOpType.max)
                            r2d[g] = r2
                        for g in pair:
                            s2 = wk.tile([128, 384], BF16, tag="s1")
                            nc.vector.tensor_tensor(out=s2[:, :srows], in0=e2d[g][:, :srows],
                                                    in1=r2d[g][:, :srows],
                                                    op=mybir.AluOpType.min)
                            s2d[g] = s2
                        h3d = {}
                        for g in pair:
                            h3 = mlp_ps.tile([128, 384], F32, tag="mlp")
                            nc.tensor.matmul(out=h3[:, :srows], lhsT=w3t[:],
                                             rhs=s2d[g][:, :srows], start=True, stop=True)
                            h3d[g] = h3
                        for g in pair:
                            h3s = wk.tile([128, 384], BF16, tag="h3s")
                            nc.vector.tensor_scalar(
                                out=h3s[:, :srows], in0=h3d[g][:, :srows],
                                scalar1=c3[:], scalar2=None, op0=mybir.AluOpType.add)
                            H3S[g].append((h3s, srows))

                    # transpose to row-major, layernorm stats, stash bf16 rows
                    for g in pair:
                        tra = tr_ps.tile([128, 384], F32, tag="tra")
                        trb = tr_ps.tile([128, 256], F32, tag="trb")
                        trs = [tra[:, 0:128], tra[:, 128:256], tra[:, 256:384],
                               trb[:, 0:128], trb[:, 128:256]]
                        for t in range(TPG):
                            sub_i, sub_off = (0, t * 128) if t < 3 else (1, (t - 3) * 128)
                            h3s, _ = H3S[g][sub_i]
                            hsl = h3s[:, sub_off:sub_off + 128]
                            nc.tensor.matmul(out=trs[t], lhsT=hsl, rhs=it[:],
                                             start=True, stop=True)
                            stt = tiny.tile([128, 6], F32, tag="bn")
                            nc.vector.bn_stats(out=stt[:], in_=trs[t])
                            nc.vector.bn_aggr(out=mvb[:, (g - gs) * TPG + t, :],
                                              in_=stt[:])
                        ha = stash.tile([128, 384], BF16, tag="ha")
                        nc.scalar.copy(out=ha[:], in_=tra[:])
                        hb = stash.tile([128, 256], BF16, tag="hb")
                        nc.scalar.copy(out=hb[:], in_=trb[:])
                        rowstash.append((ha, hb))

                # batched rstd / mu*rstd for the supergroup
                lnv = tiny.tile([128, SG * TPG], F32, tag="lnv")
                nc.scalar.activation(out=lnv[:], in_=mvb[:, :, 1],
                                     func=mybir.ActivationFunctionType.Ln,
                                     bias=eps_col[:], scale=1.0)
                rstd = tiny.tile([128, SG * TPG], F32, tag="rstd")
                nc.scalar.activation(out=rstd[:], in_=lnv[:],
                                     func=mybir.ActivationFunctionType.Exp,
                                     bias=0.0, scale=-0.5)
                nmr = tiny.tile([128, SG * TPG], F32, tag="nmr")
                nc.vector.tensor_tensor(out=nmr[:], in0=mvb[:, :, 0],
                                        in1=rstd[:], op=mybir.AluOpType.mult)
                nmrn = tiny.tile([128, SG * TPG], F32, tag="nmrn")
                nc.vector.tensor_scalar_mul(nmrn[:], nmr[:], -1.0)

                # apply layernorm, one-hot segment matmul, per-bucket mean
                for gi0 in range(0, SG, 2):
                    gis = (gi0, gi0 + 1)
                    segd, ohd = {}, {}
                    for gi in gis:
                        g = gs + gi
                        seg_t = seg_ps.tile([128, H], F32, tag="seg")
                        segd[gi] = seg_t
                        ohg = xin.tile([128, TPG, 128], BF16, tag="ohg")
                        nc.sync.dma_start(out=ohg[:],
                                          in_=o_oh[:, g * TPG:(g + 1) * TPG, :])
                        ohd[gi] = ohg
                    for t in range(TPG):
                        yd = {}
                        for gi in gis:
                            col = gi * TPG + t
                            ha, hb = rowstash[gi]
                            src_ = (ha[:, t * 128:(t + 1) * 128] if t < 3
                                    else hb[:, (t - 3) * 128:(t - 2) * 128])
                            y = wk.tile([128, 128], BF16, tag="y")
                            if t % 2 == 0:
                                nc.scalar.activation(
                                    out=y[:], in_=src_,
                                    func=mybir.ActivationFunctionType.Identity,
                                    bias=nmrn[:, col:col + 1],
                                    scale=rstd[:, col:col + 1])
                            else:
                                nc.vector.tensor_scalar(
                                    out=y[:], in0=src_,
                                    scalar1=rstd[:, col:col + 1],
                                    scalar2=nmr[:, col:col + 1],
                                    op0=mybir.AluOpType.mult,
                                    op1=mybir.AluOpType.subtract)
                            yd[gi] = y
                        for gi in gis:
                            nc.tensor.matmul(out=segd[gi][:], lhsT=ohd[gi][:, t, :],
                                             rhs=yd[gi][:],
                                             start=(t == 0), stop=(t == TPG - 1))
                    for gi in gis:
                        g = gs + gi
                        m1 = outp.tile([128, H], F32, tag="m1")
                        nc.vector.tensor_scalar(
                            out=m1[:], in0=segd[gi][:], scalar1=recip[:, g:g + 1],
                            scalar2=None, op0=mybir.AluOpType.mult)
                        # (kept on DVE: PSUM source; GPSIMD cannot read PSUM)
                        nc.gpsimd.tensor_tensor(out=m1[:], in0=m1[:], in1=gT[:],
                                                op=mybir.AluOpType.mult)
                        nc.gpsimd.tensor_tensor(out=m1[:], in0=m1[:], in1=bT[:],
                                                op=mybir.AluOpType.add)
                        nc.sync.dma_start(out=table[g * 128:(g + 1) * 128, :],
                                          in_=m1[:])

    nc.finalize()
    return nc


# ======================= host prep =======================

def _prep(e, field, edge_attr, cluster, mask_idx, idx_hr_to_lr, edge_index,
          W1, b1, W2, b2, W3, b3, ln_g, ln_b):
    bf16 = ml_dtypes.bfloat16
    cluster = np.asarray(cluster).astype(np.int64)

    # ----- node partition: sort HR rows by cluster, group into bucket groups
    order = np.argsort(cluster, kind="stable")
    csort = cluster[order]
    # boundaries of each 128-bucket group per core: group id = cluster >> 7
    gid = csort >> 7  # 0..511 global groups (64 per core)
    gstart = np.searchsorted(gid, np.arange(NCORE * GROUPS))
    gend = np.searchsorted(gid, np.arange(NCORE * GROUPS) + 1)
    glen = gend - gstart
    if glen.max() > TPG * 128:
        raise RuntimeError(f"bucket group overflow: {glen.max()} > {TPG*128}")

    perm = np.zeros((NCORE, NP_ROWS), dtype=np.int64)
    valid = np.zeros((NCORE, NP_ROWS), dtype=bool)
    o_all = np.full((NCORE, NP_ROWS), -1.0, dtype=np.float32)
    for k in range(NCORE):
        for g in range(GROUPS):
            gg = k * GROUPS + g
            n = glen[gg]
            base = g * TPG * 128
            rows = order[gstart[gg]:gend[gg]]
            perm[k, base:base + n] = rows
            valid[k, base:base + n] = True
            o_all[k, base:base + n] = (csort[gstart[gg]:gend[gg]]
                                       - (gg << 7)).astype(np.float32)

    cnt = np.bincount(cluster, minlength=N_LR).astype(np.float32)

    # per-core inputs
    e = np.asarray(e, dtype=np.float32)
    field = np.asarray(field, dtype=np.float32)
    ins = []
    w1b = np.asarray(W1, dtype=np.float32).astype(bf16)
    w2b = (LAMBDA * np.asarray(W2, dtype=np.float32)).astype(bf16)
    w3b = (LAMBDA * np.asarray(W3, dtype=np.float32)).astype(bf16)
    vecs = np.zeros((128, 8), dtype=np.float32)
    vecs[:, 0] = b1
    vecs[:, 1] = b2
    vecs[:, 2] = b3
    vecs[:, 3] = ln_g
    vecs[:, 4] = ln_b
    vrow = np.zeros((4, 128), dtype=np.float32)
    vrow[0] = np.arange(128, dtype=np.float32)
    vrow[1] = ln_g
    vrow[2] = ln_b
    ident = np.eye(128, dtype=np.float32).astype(bf16)

    # ----- edge partition: remap, drop self loops, global lexsort, coalesce
    ei0 = np.asarray(edge_index[0], dtype=np.int64)
    ei1 = np.asarray(edge_index[1], dtype=np.int64)
    idx_map = np.asarray(idx_hr_to_lr, dtype=np.int64)
    src = idx_map[ei0]
    dst = idx_map[ei1]
    keep = src != dst
    sv, dv_, eav = src[keep], dst[keep], np.asarray(edge_attr, np.float32)[keep]
    eorder = np.lexsort((dv_, sv))
    ss, ds = sv[eorder], dv_[eorder]
    ea_s = eav[eorder]
    newseg = np.empty(len(ss), dtype=bool)
    newseg[0] = True
    np.logical_or(ss[1:] != ss[:-1], ds[1:] != ds[:-1], out=newseg[1:])
    first = np.nonzero(newseg)[0]
    nseg = len(first)
    seglen = np.diff(np.append(first, len(ss)))
    seg_src = ss[first]
    seg_dst = ds[first]
    # split segments by owning core (seg_src sorted ascending)
    seg_cut = np.searchsorted(seg_src, np.arange(NCORE + 1) * BPC)

    ea_in = np.zeros((NCORE, SEGC, DE), dtype=np.float32)
    dup_vals = np.zeros((NCORE, 128, DUPC // 128, MAXD, DE), dtype=np.float32)
    dup_cnt = np.zeros((NCORE, 128, DUPC // 128), dtype=np.float32)
    dup_rows = []  # per core: local seg indices of dup segments (order = layout)
    nseg_k = np.zeros(NCORE, dtype=np.int64)
    if seglen.max() > MAXD:
        raise RuntimeError(f"dup segment too long: {seglen.max()} > {MAXD}")
    for k in range(NCORE):
        s0, s1 = seg_cut[k], seg_cut[k + 1]
        nk = s1 - s0
        nseg_k[k] = nk
        if nk > SEGC:
            raise RuntimeError(f"segment overflow core {k}: {nk} > {SEGC}")
        ea_in[k, :nk] = ea_s[first[s0:s1]]
        dl = np.nonzero(seglen[s0:s1] >= 2)[0]  # local seg ids of dups
        if len(dl) > DUPC:
            raise RuntimeError(f"dup overflow core {k}: {len(dl)} > {DUPC}")
        dup_rows.append(dl)
        for j, lseg in enumerate(dl):
            gseg = s0 + lseg
            st_e = first[gseg]
            ln = seglen[gseg]
            p, q = j % 128, j // 128
            dup_vals[k, p, q, :ln] = ea_s[st_e:st_e + ln]
            dup_cnt[k, p, q] = ln
    edge_meta = dict(nseg_k=nseg_k, seg_src=seg_src, seg_dst=seg_dst,
                     seg_cut=seg_cut, dup_rows=dup_rows)

    for k in range(NCORE):
        pk = perm[k]
        x = np.concatenate([e[pk], field[pk]], axis=1)  # (NP_ROWS, 256)
        x[~valid[k]] = 0.0
        xT = np.ascontiguousarray(x.T).astype(bf16)
        o_cm = np.ascontiguousarray(
            o_all[k].reshape(NTILES, 128).T).astype(np.float32)
        o_oh = np.ascontiguousarray(
            (o_cm[:, :, None] == np.arange(128, dtype=np.float32)[None, None, :])
        ).astype(ml_dtypes.bfloat16)
        cnt_k = cnt[k * BPC:(k + 1) * BPC]
        cnt_cm = np.ascontiguousarray(
            cnt_k.reshape(GROUPS, 128).T).astype(np.float32)
        ins.append({
            "xT": xT, "o_cm": o_cm, "cnt_cm": cnt_cm, "o_oh": o_oh,
            "w1": w1b, "w2": w2b, "w3": w3b,
            "vecs": vecs, "vrow": vrow, "ident": ident,
            "ea_in": ea_in[k], "dup_vals": dup_vals[k], "dup_cnt": dup_cnt[k],
        })
    return ins, edge_meta


# ======================= entry point =======================

def kernel(e, field, edge_attr, cluster, mask_idx, idx_hr_to_lr, edge_index,
           W1, b1, W2, b2, W3, b3, ln_g, ln_b):
    if "nc" not in _COMPILED:
        _COMPILED["nc"] = build_graph()
    nc = _COMPILED["nc"]

    ins, em = _prep(e, field, edge_attr, cluster, mask_idx, idx_hr_to_lr,
                    edge_index, W1, b1, W2, b2, W3, b3, ln_g, ln_b)

    rr = run_bass_kernel_spmd(nc, ins, list(range(NCORE)))
    _COMPILED["last_exec_time_ns"] = rr.exec_time_ns
    res = rr.results

    # ----- unshard: field_lr via mask gather over the per-core tables
    mask_idx = np.asarray(mask_idx, dtype=np.int64)
    field_lr = np.empty((NMASK, H), dtype=np.float32)
    mcut = np.searchsorted(mask_idx, np.arange(NCORE + 1) * BPC)
    for k in range(NCORE):
        mk = mask_idx[mcut[k]:mcut[k + 1]] - k * BPC
        field_lr[mcut[k]:mcut[k + 1]] = res[k]["table"][mk]

    # ----- unshard: ei / ea
    nseg_k = em["nseg_k"]
    nseg = int(nseg_k.sum())
    ei = np.full((2, E), -1, dtype=np.int32)
    ei[0, :nseg] = em["seg_src"]
    ei[1, :nseg] = em["seg_dst"]
    ea = np.zeros((E, DE), dtype=np.float32)
    off = 0
    for k in range(NCORE):
        nk = int(nseg_k[k])
        ea_k = res[k]["ea_out"][:nk].copy()
        dl = em["dup_rows"][k]
        if len(dl):
            dm = res[k]["dup_out"]  # (128, DUPC//128, DE)
            j = np.arange(len(dl))
            ea_k[dl] = dm[j % 128, j // 128]
        ea[off:off + nk] = ea_k
        off += nk

    return field_lr, ei, ea
